# revision 11
# baseline (speedup 1.0000x reference)
"""Self-attention kernel for Trainium2 (Bass/Tile), 8-core SPMD.

Problem: X [4, 4096, 512] f32
  S = X @ X^T per batch     [4, 4096, 4096]
  W = softmax(S, axis=-1)
  Y = W @ X                 [4, 4096, 512]

Sharding: data-parallel over batch (4 batches x 2 cores) + query-sequence
parallel within a batch (each core owns 2048 queries, sees all 4096 keys).
Host rolls each batch's key axis per core so the core's queries always sit
at rows/cols 0..2047 — the SPMD program is identical on all 8 cores and the
softmax reduction over keys is permutation-invariant.

Per-core device program (full attention, no shortcuts):
  - X^T (d-major) and X (n-major) resident in SBUF as float32r
    (f32r = fp32 streamed at bf16 rate through the PE with 12-bit-mantissa
    rounding of operands; PSUM accumulation is full fp32)
  - per 128-query block: scores via PE (f32r), row-max on DVE, exp on ACT
    (with fused row-sum accumulation), 128x128 PE transposes of the
    probability block, P^T @ X via PE (f32r), normalize by 1/l, DMA out.
"""

import numpy as np

import concourse.bass as bass  # noqa: F401  (registers bass types)
import concourse.mybir as mybir
import concourse.tile as tile
from concourse import bacc
from concourse.bass_utils import run_bass_kernel_spmd
from concourse.masks import make_identity

F32 = mybir.dt.float32
F32R = mybir.dt.float32r
AX = mybir.AxisListType.X

P = 128          # partitions / query block
D = 512          # head dim
DC = D // P      # 4 d-chunks (contraction for scores)
NK = 4096        # keys per batch
NQ = 2048        # queries per core
NW = 512         # matmul moving width / PSUM bank width (fp32)
KT = NK // NW    # 8 key tiles per score row-block
KC = NK // P     # 32 key chunks (PV contraction)
NB = NQ // P     # 16 query blocks per core
N_CORES = 8
B = 4

_cached = None  # (nc, ...) build once per process


def _build_program():
    nc = bacc.Bacc("TRN2", target_bir_lowering=False, debug=False)
    xt_d = nc.dram_tensor("xt", [D, NK], F32, kind="ExternalInput").ap()
    xn_d = nc.dram_tensor("xn", [NK, D], F32, kind="ExternalInput").ap()
    o_d = nc.dram_tensor("o", [NQ, D], F32, kind="ExternalOutput").ap()
    o_tiles = o_d.rearrange("(t p) d -> t p d", p=P)

    with tile.TileContext(nc) as tc:
        with tc.tile_pool(name="consts", bufs=1) as consts, \
             tc.tile_pool(name="pblk", bufs=3) as pblk, \
             tc.tile_pool(name="ptblk", bufs=1) as ptblk, \
             tc.tile_pool(name="stats", bufs=3) as stats, \
             tc.tile_pool(name="outp", bufs=2) as outp, \
             tc.tile_pool(name="ps_s", bufs=4, space="PSUM") as ps_s, \
             tc.tile_pool(name="ps_t", bufs=2, space="PSUM") as ps_t, \
             tc.tile_pool(name="ps_pv", bufs=2, space="PSUM") as ps_pv:

            xt_s = consts.tile([P, DC, NK], F32R)   # X^T, d on partitions
            xn_s = consts.tile([P, KC, D], F32R)    # X, keys on partitions

            ident_f = consts.tile([P, P], F32)
            make_identity(nc, ident_f)
            ident = consts.tile([P, P], F32R)
            nc.vector.tensor_copy(ident, ident_f)

            # Input DMA: first-needed-first, in 512-col slivers so each score
            # tile's dependency releases as soon as its slice lands. xn
            # follows (first needed by PV of block 0).
            for j in range(KT):
                for c in range(DC):
                    nc.sync.dma_start(
                        xt_s[:, c, j * NW:(j + 1) * NW],
                        xt_d[c * P:(c + 1) * P, j * NW:(j + 1) * NW].bitcast(F32R))
            xn_r = xn_d.rearrange("(t p) d -> p t d", p=P)
            for g in range(8):
                nc.sync.dma_start(
                    xn_s[:, g * (KC // 8):(g + 1) * (KC // 8), :],
                    xn_r[:, g * (KC // 8):(g + 1) * (KC // 8), :].bitcast(F32R))

            def new_block():
                return {
                    "p_s": pblk.tile([P, KT, NW], F32R, name="p_s", tag="p_s"),
                    "mparts": stats.tile([P, KT], F32, name="mparts", tag="mparts"),
                    "negm": stats.tile([P, 1], F32, name="negm", tag="negm"),
                    "lparts": stats.tile([P, KT], F32, name="lparts", tag="lparts"),
                }

            def s_tile(qb, j, blk):
                """One 128x512 score tile: 4 accumulating MMs + copy + max."""
                s_ps = ps_s.tile([P, NW], F32)
                for c in range(DC):
                    nc.tensor.matmul(
                        s_ps,
                        xt_s[:, c, qb * P:(qb + 1) * P],
                        xt_s[:, c, j * NW:(j + 1) * NW],
                        start=(c == 0), stop=(c == DC - 1))
                nc.vector.tensor_copy(blk["p_s"][:, j, :], s_ps)
                nc.vector.reduce_max(blk["mparts"][:, j:j + 1], s_ps, axis=AX)

            def exp_block(blk):
                p_s, negm = blk["p_s"], blk["negm"]
                nc.vector.reduce_max(negm, blk["mparts"], axis=AX, negate=True)
                for j in range(KT):
                    nc.scalar.activation(
                        p_s[:, j, :], p_s[:, j, :],
                        mybir.ActivationFunctionType.Exp,
                        bias=negm, scale=1.0,
                        accum_out=blk["lparts"][:, j:j + 1])

            def s_phase(qb):
                """Scores + softmax numerator for query block qb."""
                blk = new_block()
                for j in range(KT):
                    s_tile(qb, j, blk)
                exp_block(blk)
                return blk

            def o_phase(qb, blk):
                """Transpose P, P^T @ X, normalize, store."""
                p_s, lparts = blk["p_s"], blk["lparts"]
                pt_s = ptblk.tile([P, KC, P], F32R)
                for g in range(KT):
                    t_ps = ps_t.tile([P, 4, P], F32R)
                    for cc in range(4):
                        nc.tensor.transpose(
                            t_ps[:, cc, :],
                            p_s[:, g, cc * P:(cc + 1) * P],
                            ident)
                    nc.scalar.copy(out=pt_s[:, 4 * g:4 * (g + 1), :], in_=t_ps)
                l_sum = stats.tile([P, 1], F32)
                rl = stats.tile([P, 1], F32)
                nc.vector.reduce_sum(l_sum, lparts, axis=AX)
                nc.vector.reciprocal(rl, l_sum)
                pv_ps = ps_pv.tile([P, NW], F32)
                for k in range(KC):
                    nc.tensor.matmul(
                        pv_ps, pt_s[:, k, :], xn_s[:, k, :],
                        start=(k == 0), stop=(k == KC - 1))
                o_s = outp.tile([P, NW], F32)
                nc.vector.tensor_scalar_mul(o_s, pv_ps, rl)
                nc.sync.dma_start(o_tiles[qb], o_s)

            # Warmup: the first WARM blocks' score tiles interleave j-outer,
            # so the PE consumes each freshly-DMA'd xt sliver WARM times
            # while the next sliver streams in. Afterwards, software-
            # pipelined emission keeps the PE stream dense:
            #   ... S_qb | T_{qb-2} PV_{qb-2} | S_{qb+1} | T_{qb-1} ...
            WARM = 2
            warm_blks = [new_block() for _ in range(WARM)]
            for j in range(KT):
                for qb in range(WARM):
                    s_tile(qb, j, warm_blks[qb])
            for blk in warm_blks:
                exp_block(blk)

            pending = [(qb, warm_blks[qb]) for qb in range(WARM)]
            for qb in range(WARM, NB):
                cur = s_phase(qb)
                o_phase(*pending.pop(0))
                pending.append((qb, cur))
            for item in pending:
                o_phase(*item)

    nc.compile()
    return nc


def _get_program():
    global _cached
    if _cached is None:
        _cached = _build_program()
    return _cached


def _make_in_maps(X):
    in_maps = []
    for b in range(B):
        Xb = np.ascontiguousarray(X[b], dtype=np.float32)
        for h in range(2):
            qoff = h * NQ
            if qoff == 0:
                rolled = Xb
            else:
                rolled = np.ascontiguousarray(
                    np.concatenate([Xb[qoff:], Xb[:qoff]], axis=0))
            in_maps.append({
                "xn": rolled,
                "xt": np.ascontiguousarray(rolled.T),
            })
    return in_maps


def run(X, trace=False, trace_kwargs=None):
    """Run the 8-core kernel on full X [4, 4096, 512]; returns (Y, results)."""
    X = np.asarray(X)
    assert X.shape == (B, NK, D), X.shape
    nc = _get_program()
    in_maps = _make_in_maps(X)
    res = run_bass_kernel_spmd(
        nc, in_maps, core_ids=list(range(N_CORES)),
        trace=trace, **(trace_kwargs or {}))
    out = np.empty((B, NK, D), dtype=np.float32)
    for b in range(B):
        for h in range(2):
            out[b, h * NQ:(h + 1) * NQ] = res.results[2 * b + h]["o"]
    return out, res


def kernel(X):
    out, _ = run(X)
    return out


# revision 18
# speedup vs baseline: 241.6101x; 241.6101x over previous
"""Self-attention kernel for Trainium2 (Bass/Tile), 8-core SPMD.

Problem: X [4, 4096, 512] f32
  S = X @ X^T per batch     [4, 4096, 4096]
  W = softmax(S, axis=-1)
  Y = W @ X                 [4, 4096, 512]

Sharding: data-parallel over batch (4 batches x 2 cores) + query-sequence
parallel within a batch (each core owns 2048 queries, sees all 4096 keys).
Host rolls each batch's key axis per core so the core's queries always sit
at rows/cols 0..2047 — the SPMD program is identical on all 8 cores and the
softmax reduction over keys is permutation-invariant.

Per-core device program (full attention, no shortcuts):
  - X^T (d-major) and X (n-major) resident in SBUF as float32r
    (f32r = fp32 streamed at bf16 rate through the PE with 12-bit-mantissa
    rounding of operands; PSUM accumulation is full fp32)
  - per 128-query block: scores via PE (f32r), row-max on DVE, exp on ACT
    (with fused row-sum accumulation), 128x128 PE transposes of the
    probability block, P^T @ X via PE (f32r), normalize by 1/l, DMA out.
"""

import numpy as np

import concourse.bass as bass  # noqa: F401  (registers bass types)
import concourse.mybir as mybir
import concourse.tile as tile
from concourse import bacc
from concourse.bass_utils import run_bass_kernel_spmd
from concourse.masks import make_identity

F32 = mybir.dt.float32
F32R = mybir.dt.float32r
AX = mybir.AxisListType.X

P = 128          # partitions / query block
D = 512          # head dim
DC = D // P      # 4 d-chunks (contraction for scores)
NK = 4096        # keys per batch
NQ = 2048        # queries per core
NW = 512         # matmul moving width / PSUM bank width (fp32)
KT = NK // NW    # 8 key tiles per score row-block
KC = NK // P     # 32 key chunks (PV contraction)
NB = NQ // P     # 16 query blocks per core
N_CORES = 8
B = 4

_cached = None  # (nc, ...) build once per process


def _build_program():
    nc = bacc.Bacc("TRN2", target_bir_lowering=False, debug=False)
    xt_d = nc.dram_tensor("xt", [D, NK], F32, kind="ExternalInput").ap()
    xn_d = nc.dram_tensor("xn", [NK, D], F32, kind="ExternalInput").ap()
    o_d = nc.dram_tensor("o", [NQ, D], F32, kind="ExternalOutput").ap()
    o_tiles = o_d.rearrange("(t p) d -> t p d", p=P)

    with tile.TileContext(nc) as tc:
        with tc.tile_pool(name="consts", bufs=1) as consts, \
             tc.tile_pool(name="pblk", bufs=3) as pblk, \
             tc.tile_pool(name="ptblk", bufs=1) as ptblk, \
             tc.tile_pool(name="stats", bufs=3) as stats, \
             tc.tile_pool(name="outp", bufs=1) as outp, \
             tc.tile_pool(name="ps_s", bufs=5, space="PSUM") as ps_s, \
             tc.tile_pool(name="ps_t", bufs=2, space="PSUM") as ps_t, \
             tc.tile_pool(name="ps_pv", bufs=1, space="PSUM") as ps_pv:

            xt_s = consts.tile([P, DC, NK], F32R)   # X^T, d on partitions
            xn_s = consts.tile([P, KC, D], F32R)    # X, keys on partitions

            # identity staging tile borrows a p_s slot (released on reuse)
            ident_f = pblk.tile([P, P], F32, name="ident_f", tag="p_s")
            make_identity(nc, ident_f)
            ident = consts.tile([P, P], F32R)
            nc.vector.tensor_copy(ident, ident_f)

            # Input DMA: first-needed-first, in 512-col slivers so each score
            # tile's dependency releases as soon as its slice lands. xn
            # follows (first needed by PV of block 0).
            xt_r = xt_d.rearrange("(c p) n -> p c n", p=P)
            for c in range(DC):
                nc.sync.dma_start(
                    xt_s[:, c, 0:NW],
                    xt_d[c * P:(c + 1) * P, 0:NW].bitcast(F32R))
            for j in range(1, KT):
                nc.sync.dma_start(
                    xt_s[:, :, j * NW:(j + 1) * NW],
                    xt_r[:, :, j * NW:(j + 1) * NW].bitcast(F32R))
            xn_r = xn_d.rearrange("(t p) d -> p t d", p=P)
            for g in range(8):
                nc.sync.dma_start(
                    xn_s[:, g * (KC // 8):(g + 1) * (KC // 8), :],
                    xn_r[:, g * (KC // 8):(g + 1) * (KC // 8), :].bitcast(F32R))

            def new_block():
                return {
                    "p_s": pblk.tile([P, KT, NW], F32R, name="p_s", tag="p_s"),
                    "mparts": stats.tile([P, KT], F32, name="mparts", tag="mparts"),
                    "negm": stats.tile([P, 1], F32, name="negm", tag="negm"),
                    "lparts": stats.tile([P, KT], F32, name="lparts", tag="lparts"),
                }

            def s_tile(qb, j, blk):
                """One 128x512 score tile: 4 accumulating MMs + copy + max."""
                s_ps = ps_s.tile([P, NW], F32)
                for c in range(DC):
                    nc.tensor.matmul(
                        s_ps,
                        xt_s[:, c, qb * P:(qb + 1) * P],
                        xt_s[:, c, j * NW:(j + 1) * NW],
                        start=(c == 0), stop=(c == DC - 1))
                nc.vector.tensor_copy(blk["p_s"][:, j, :], s_ps)
                nc.vector.reduce_max(blk["mparts"][:, j:j + 1], s_ps, axis=AX)

            def exp_block(blk):
                p_s, negm = blk["p_s"], blk["negm"]
                nc.vector.reduce_max(negm, blk["mparts"], axis=AX, negate=True)
                for j in range(KT):
                    nc.scalar.activation(
                        p_s[:, j, :], p_s[:, j, :],
                        mybir.ActivationFunctionType.Exp,
                        bias=negm, scale=1.0,
                        accum_out=blk["lparts"][:, j:j + 1])

            def s_phase(qb):
                """Scores + softmax numerator for query block qb."""
                blk = new_block()
                for j in range(KT):
                    s_tile(qb, j, blk)
                exp_block(blk)
                return blk

            def o_phase(qb, blk):
                """Transpose P, P^T @ X, normalize, store."""
                p_s, lparts = blk["p_s"], blk["lparts"]
                pt_s = ptblk.tile([P, KC, P], F32R)
                for g in range(KT):
                    t_ps = ps_t.tile([P, 4, P], F32R)
                    for cc in range(4):
                        nc.tensor.transpose(
                            t_ps[:, cc, :],
                            p_s[:, g, cc * P:(cc + 1) * P],
                            ident)
                    nc.scalar.copy(out=pt_s[:, 4 * g:4 * (g + 1), :], in_=t_ps)
                l_sum = stats.tile([P, 1], F32)
                rl = stats.tile([P, 1], F32)
                nc.vector.reduce_sum(l_sum, lparts, axis=AX)
                nc.vector.reciprocal(rl, l_sum)
                pv_ps = ps_pv.tile([P, NW], F32)
                for k in range(KC):
                    nc.tensor.matmul(
                        pv_ps, pt_s[:, k, :], xn_s[:, k, :],
                        start=(k == 0), stop=(k == KC - 1))
                o_s = outp.tile([P, NW], F32)
                nc.vector.tensor_scalar_mul(o_s, pv_ps, rl)
                nc.sync.dma_start(o_tiles[qb], o_s)

            # Warmup: the first WARM blocks' score tiles interleave j-outer,
            # so the PE consumes each freshly-DMA'd xt sliver WARM times
            # while the next sliver streams in. Afterwards, software-
            # pipelined emission keeps the PE stream dense:
            #   ... S_qb | T_{qb-2} PV_{qb-2} | S_{qb+1} | T_{qb-1} ...
            WARM = 2
            warm_blks = [new_block() for _ in range(WARM)]
            for j in range(KT):
                for qb in range(WARM):
                    s_tile(qb, j, warm_blks[qb])
            for blk in warm_blks:
                exp_block(blk)

            pending = [(qb, warm_blks[qb]) for qb in range(WARM)]
            for qb in range(WARM, NB):
                cur = s_phase(qb)
                o_phase(*pending.pop(0))
                pending.append((qb, cur))
            for item in pending:
                o_phase(*item)

    nc.compile()
    return nc


def _get_program():
    global _cached
    if _cached is None:
        _cached = _build_program()
    return _cached


def _make_in_maps(X):
    in_maps = []
    for b in range(B):
        Xb = np.ascontiguousarray(X[b], dtype=np.float32)
        for h in range(2):
            qoff = h * NQ
            if qoff == 0:
                rolled = Xb
            else:
                rolled = np.ascontiguousarray(
                    np.concatenate([Xb[qoff:], Xb[:qoff]], axis=0))
            in_maps.append({
                "xn": rolled,
                "xt": np.ascontiguousarray(rolled.T),
            })
    return in_maps


def run(X, trace=False, trace_kwargs=None):
    """Run the 8-core kernel on full X [4, 4096, 512]; returns (Y, results)."""
    X = np.asarray(X)
    assert X.shape == (B, NK, D), X.shape
    nc = _get_program()
    in_maps = _make_in_maps(X)
    res = run_bass_kernel_spmd(
        nc, in_maps, core_ids=list(range(N_CORES)),
        trace=trace, **(trace_kwargs or {}))
    out = np.empty((B, NK, D), dtype=np.float32)
    for b in range(B):
        for h in range(2):
            out[b, h * NQ:(h + 1) * NQ] = res.results[2 * b + h]["o"]
    return out, res


def kernel(X):
    out, _ = run(X)
    return out
